# revision 16
# baseline (speedup 1.0000x reference)
"""Trainium2 Bass kernel for nn_DYAN: 2-round reweighted FISTA sparse coding.

Strategy (data-parallel over samples, 8 cores):
  - Each core gets 16 of 128 samples: Y [T=36, F=800] (F = 16 samples x 50 dims).
  - On device (per core, fully replicated small state):
      * Build dictionary D [36, 161->192 padded] from rho/theta via ACT Ln/Exp/Sin.
      * DtD via PE; spectral norm L via 6 matrix squarings + Rayleigh quotient.
      * A = I - DtD/L; b = DtY/L.
      * 2 x 100 FISTA iterations. Momentum is carried in a streamed variable
        w_k = s_{k-1} x_{k-1} - x_k  (so y_k = -(1+tt_{k-1}) w_k), which makes the
        per-iteration elementwise work 3 two-tensor passes + 2 ACT passes.
        The convergence check of the reference never triggers for 100 iters
        (verified: min ||x-x_new||/Np ~ 2e-3 >> 1e-5), so no early exit logic.
      * Reweighting between rounds computed on device.
      * Outputs C [161,800], R = D @ C [36,800], D [36,161], L (debug).
All matmuls fp32 (float32r is tf32-like and fails the 200-iteration error
amplification; bf16 much worse).
"""
import os
import numpy as np

import concourse.bass as bass
import concourse.tile as tile
from concourse import bacc, mybir
from concourse.bass_utils import run_bass_kernel_spmd

F32 = mybir.dt.float32
OP = mybir.AluOpType
AF = mybir.ActivationFunctionType

T = 36
NP_ = 161
PAD = 192
NS = 128
DD = 50
LAM = 0.1
N_CORES = 8
NSH = NS // N_CORES          # 16 samples per core
F = NSH * DD                 # 800
FC = F // 2                  # 400 (chunk)
N_ITERS = int(os.environ.get("KBENCH_ITERS", "100"))
N_REPEAT = int(os.environ.get("KBENCH_REPEAT", "1"))
N_SQUARINGS = 6


LAST_EXEC_NS = None
LAST_RESULTS = None


def _t_sequence(n):
    """Replicate the reference's fp32 t/tt sequence on host."""
    t = np.float32(1.0)
    tts = []
    for _ in range(n):
        t_new = (np.float32(1.0) + np.float32(np.sqrt(np.float32(1.0) + np.float32(4.0) * t * t))) / np.float32(2.0)
        t_new = np.float32(t_new)
        tt = np.float32((t - np.float32(1.0)) / t_new)
        tts.append(tt)
        t = t_new
    return tts


def _build_module():
    nc = bacc.Bacc("TRN2", target_bir_lowering=False, debug=False)

    Y_in = nc.dram_tensor("Y_in", [T, F], F32, kind="ExternalInput").ap()
    rho_in = nc.dram_tensor("rho_in", [80, 1], F32, kind="ExternalInput").ap()
    th_in = nc.dram_tensor("th_in", [80, 1], F32, kind="ExternalInput").ap()
    id_in = nc.dram_tensor("id_in", [PAD, PAD], F32, kind="ExternalInput").ap()
    r0_in = nc.dram_tensor("r0_in", [PAD, 1], F32, kind="ExternalInput").ap()
    C_out = nc.dram_tensor("C_out", [NP_, F], F32, kind="ExternalOutput").ap()
    R_out = nc.dram_tensor("R_out", [T, F], F32, kind="ExternalOutput").ap()
    D_out = nc.dram_tensor("D_out", [T, NP_], F32, kind="ExternalOutput").ap()
    L_out = nc.dram_tensor("L_out", [1, 1], F32, kind="ExternalOutput").ap()
    C1_out = nc.dram_tensor("C1_out", [NP_, F], F32, kind="ExternalOutput").ap()
    WL_out = nc.dram_tensor("WL_out", [NP_, F], F32, kind="ExternalOutput").ap()

    tts = _t_sequence(N_ITERS)
    # alpha_k = -(1+tt_{k-1}) used at iteration k; s_k = tt_k/(1+tt_k)
    alphas = [float(-(np.float32(1.0) + tt)) for tt in tts]
    ss = [float(np.float32(tt / (np.float32(1.0) + tt))) for tt in tts]

    with tile.TileContext(nc) as tc:
        with tc.tile_pool(name="cst", bufs=1) as CP, \
             tc.tile_pool(name="state", bufs=3) as SP, \
             tc.tile_pool(name="tmp", bufs=2) as TP:

            # ---------------- loads ----------------
            Y_sb = CP.tile([T, F], F32, tag="Y")
            nc.sync.dma_start(Y_sb[:], Y_in[:])
            rho_t = CP.tile([80, 1], F32, tag="rho")
            th_t = CP.tile([80, 1], F32, tag="th")
            nc.sync.dma_start(rho_t[:], rho_in[:])
            nc.sync.dma_start(th_t[:], th_in[:])
            ID_lo = CP.tile([128, PAD], F32, tag="idlo")
            ID_hi = CP.tile([64, PAD], F32, tag="idhi")
            nc.sync.dma_start(ID_lo[:], id_in[0:128, :])
            nc.sync.dma_start(ID_hi[:], id_in[128:PAD, :])
            r0_lo = CP.tile([128, 1], F32, tag="r0lo")
            r0_hi = CP.tile([64, 1], F32, tag="r0hi")
            nc.sync.dma_start(r0_lo[:], r0_in[0:128, :])
            nc.sync.dma_start(r0_hi[:], r0_in[128:PAD, :])

            ones_row = CP.tile([1, 128], F32, tag="onesrow")
            nc.vector.memset(ones_row[:], 1.0)
            ones_lo = CP.tile([128, 1], F32, tag="oneslo")
            nc.vector.memset(ones_lo[:], 1.0)
            ones_hi = CP.tile([64, 1], F32, tag="oneshi")  # masks padding rows
            nc.vector.memset(ones_hi[:], 0.0)
            nc.vector.memset(ones_hi[0:33, :], 1.0)
            hpi = CP.tile([80, 1], F32, tag="hpi")
            nc.vector.memset(hpi[:], float(np.pi / 2))

            with tc.tile_pool(name="pss", bufs=1, space="PSUM") as PS:
                # ---------------- build Dt ([pole-part, t-free]) ----------------
                io_i = TP.tile([80, T], mybir.dt.int32, tag="ioi")
                nc.gpsimd.iota(io_i[:], pattern=[[1, T]], base=0, channel_multiplier=0)
                io_f = CP.tile([80, T], F32, tag="iof")
                nc.vector.tensor_copy(io_f[:], io_i[:])
                nc.vector.tensor_scalar(rho_t[:], rho_t[:], 0.8, 1.1, OP.max, OP.min)
                nc.vector.tensor_scalar(th_t[:], th_t[:], 0.1, float(np.pi), OP.max, OP.min)
                lnr = TP.tile([80, 1], F32, tag="lnr")
                nc.scalar.activation(lnr[:], rho_t[:], AF.Ln)
                parg = TP.tile([80, T], F32, tag="parg")
                nc.vector.tensor_scalar(parg[:], io_f[:], lnr[:], None, OP.mult)
                powr = TP.tile([80, T], F32, tag="powr")
                nc.scalar.activation(powr[:], parg[:], AF.Exp)
                # i*theta can reach 35*pi; ACT Sin needs args near zero.
                # Cody-Waite: red = ang - k*(2pi) in 3 exact pieces, then wrap
                # into [-pi, pi] (also +pi/2 shifted for cos).
                CW1, CW2, CW3 = 6.28125, 0.0019350051879882812, 3.019916050561733e-07
                ang = TP.tile([80, T], F32, tag="ang")
                nc.vector.tensor_scalar(ang[:], io_f[:], th_t[:], None, OP.mult)
                m_t = TP.tile([80, T], F32, tag="mt")
                nc.vector.tensor_scalar(m_t[:], ang[:], float(1.0 / (2 * np.pi)), None, OP.mult)
                k_i = TP.tile([80, T], mybir.dt.int32, tag="ki")
                nc.vector.tensor_copy(k_i[:], m_t[:])
                k_f = TP.tile([80, T], F32, tag="kf")
                nc.vector.tensor_copy(k_f[:], k_i[:])
                red = TP.tile([80, T], F32, tag="red")
                nc.vector.cody_waite_cascade(red[:], ang[:], k_f[:], CW1, CW2, CW3)
                sarg = TP.tile([80, T], F32, tag="sarg")
                nc.vector.add_range_wrap(sarg[:], red[:], 0.0, float(np.pi), float(2 * np.pi))
                sinv = TP.tile([80, T], F32, tag="sinv")
                nc.scalar.activation(sinv[:], sarg[:], AF.Sin)
                carg = TP.tile([80, T], F32, tag="carg")
                nc.vector.add_range_wrap(carg[:], red[:], float(np.pi / 2), float(np.pi), float(2 * np.pi))
                cosv = TP.tile([80, T], F32, tag="cosv")
                nc.scalar.activation(cosv[:], carg[:], AF.Sin)
                W1t = TP.tile([80, T], F32, tag="w1t")
                nc.vector.tensor_mul(W1t[:], powr[:], cosv[:])
                W2t = TP.tile([80, T], F32, tag="w2t")
                nc.vector.tensor_mul(W2t[:], powr[:], sinv[:])

                Dt_lo = CP.tile([128, T], F32, tag="dtlo")
                Dt_hi = CP.tile([64, T], F32, tag="dthi")
                nc.vector.memset(Dt_lo[0:1, :], 1.0)
                nc.sync.dma_start(Dt_lo[1:81, :], W1t[:])
                nc.sync.dma_start(Dt_lo[81:128, :], W2t[0:47, :])
                nc.vector.memset(Dt_hi[:], 0.0)
                nc.sync.dma_start(Dt_hi[0:33, :], W2t[47:80, :])

                # ---------------- D ([t-part, pole-free]) via PE transpose ----------------
                D_sb = CP.tile([T, PAD], F32, tag="D")
                tps0 = PS.tile([T, 128], F32, tag="tp0")
                nc.tensor.transpose(tps0[:], Dt_lo[:], ID_lo[:, 0:128])
                nc.scalar.activation(D_sb[:, 0:128], tps0[:], AF.Copy)
                tps1 = PS.tile([T, 64], F32, tag="tp1")
                nc.tensor.transpose(tps1[:], Dt_hi[:], ID_lo[0:64, 0:64])
                nc.scalar.activation(D_sb[:, 128:PAD], tps1[:], AF.Copy)
                nc.sync.dma_start(D_out[:], D_sb[:, 0:NP_])

                # ---------------- DtD ----------------
                DtD_lo = CP.tile([128, PAD], F32, tag="dtdlo")
                DtD_hi = CP.tile([64, PAD], F32, tag="dtdhi")
                dps0 = PS.tile([128, PAD], F32, tag="sq0")
                nc.tensor.matmul(dps0[:], D_sb[:, 0:128], D_sb[:], start=True, stop=True)
                nc.scalar.activation(DtD_lo[:], dps0[:], AF.Copy)
                dps1 = PS.tile([64, PAD], F32, tag="sq1")
                nc.tensor.matmul(dps1[:], D_sb[:, 128:PAD], D_sb[:], start=True, stop=True)
                nc.scalar.activation(DtD_hi[:], dps1[:], AF.Copy)

                # ---------------- L via repeated squaring + Rayleigh ----------------
                Mc_lo, Mc_hi = DtD_lo, DtD_hi
                for j in range(N_SQUARINGS):
                    sp0 = PS.tile([128, PAD], F32, tag="sq0")
                    nc.tensor.matmul(sp0[:], Mc_lo[:, 0:128], Mc_lo[:], start=True, stop=False)
                    nc.tensor.matmul(sp0[:], Mc_hi[:, 0:128], Mc_hi[:], start=False, stop=True)
                    sp1 = PS.tile([64, PAD], F32, tag="sq1")
                    nc.tensor.matmul(sp1[:], Mc_lo[:, 128:PAD], Mc_lo[:], start=True, stop=False)
                    nc.tensor.matmul(sp1[:], Mc_hi[:, 128:PAD], Mc_hi[:], start=False, stop=True)
                    Mn_lo = SP.tile([128, PAD], F32, tag="msqlo")
                    Mn_hi = SP.tile([64, PAD], F32, tag="msqhi")
                    nc.scalar.activation(Mn_lo[:], sp0[:], AF.Copy)
                    nc.scalar.activation(Mn_hi[:], sp1[:], AF.Copy)
                    # renormalize by abs-sum to keep fp32 range (direction invariant)
                    rs_lo = TP.tile([128, 1], F32, tag="rslo")
                    nc.vector.tensor_reduce(rs_lo[:], Mn_lo[:], mybir.AxisListType.X, OP.add, apply_absolute_value=True)
                    rs_hi = TP.tile([64, 1], F32, tag="rshi")
                    nc.vector.tensor_reduce(rs_hi[:], Mn_hi[:], mybir.AxisListType.X, OP.add, apply_absolute_value=True)
                    nps = PS.tile([1, 1], F32, tag="dot")
                    nc.tensor.matmul(nps[:], ones_lo[:], rs_lo[:], start=True, stop=False)
                    nc.tensor.matmul(nps[:], ones_lo[0:64, :], rs_hi[:], start=False, stop=True)
                    ssum = TP.tile([1, 1], F32, tag="ssum")
                    nc.scalar.activation(ssum[:], nps[:], AF.Copy)
                    rsum = TP.tile([1, 1], F32, tag="rsum")
                    nc.vector.reciprocal(rsum[:], ssum[:])
                    bps = PS.tile([128, 1], F32, tag="bc1")
                    nc.tensor.matmul(bps[:], ones_row[:], rsum[:], start=True, stop=True)
                    sc_col = TP.tile([128, 1], F32, tag="sccol")
                    nc.scalar.activation(sc_col[:], bps[:], AF.Copy)
                    nc.vector.tensor_scalar(Mn_lo[:], Mn_lo[:], sc_col[:], None, OP.mult)
                    nc.vector.tensor_scalar(Mn_hi[:], Mn_hi[:], sc_col[0:64, :], None, OP.mult)
                    Mc_lo, Mc_hi = Mn_lo, Mn_hi

                def mv(lhsT_lo, lhsT_hi, v_lo, v_hi, tag):
                    """out [PAD,1] = M @ v (M symmetric, stored as lo/hi lhsT tiles)."""
                    p0 = PS.tile([128, 1], F32, tag="mv0")
                    nc.tensor.matmul(p0[:], lhsT_lo[:, 0:128], v_lo[:], start=True, stop=False)
                    nc.tensor.matmul(p0[:], lhsT_hi[:, 0:128], v_hi[:], start=False, stop=True)
                    p1 = PS.tile([64, 1], F32, tag="mv1")
                    nc.tensor.matmul(p1[:], lhsT_lo[:, 128:PAD], v_lo[:], start=True, stop=False)
                    nc.tensor.matmul(p1[:], lhsT_hi[:, 128:PAD], v_hi[:], start=False, stop=True)
                    o_lo = TP.tile([128, 1], F32, tag=tag + "lo")
                    o_hi = TP.tile([64, 1], F32, tag=tag + "hi")
                    nc.scalar.activation(o_lo[:], p0[:], AF.Copy)
                    nc.scalar.activation(o_hi[:], p1[:], AF.Copy)
                    return o_lo, o_hi

                c_lo, c_hi = mv(Mc_lo, Mc_hi, r0_lo, r0_hi, "cv")
                q_lo, q_hi = mv(DtD_lo, DtD_hi, c_lo, c_hi, "qv")

                def dot(a_lo, a_hi, b_lo, b_hi, tag):
                    p = PS.tile([1, 1], F32, tag="dot")
                    nc.tensor.matmul(p[:], a_lo[:], b_lo[:], start=True, stop=False)
                    nc.tensor.matmul(p[:], a_hi[:], b_hi[:], start=False, stop=True)
                    o = TP.tile([1, 1], F32, tag=tag)
                    nc.scalar.activation(o[:], p[:], AF.Copy)
                    return o

                num = dot(c_lo, c_hi, q_lo, q_hi, "num")   # c^T DtD c
                den = dot(c_lo, c_hi, c_lo, c_hi, "den")   # c^T c
                rnum = TP.tile([1, 1], F32, tag="rnum")
                nc.vector.reciprocal(rnum[:], num[:])
                Linv_sb = CP.tile([1, 1], F32, tag="linv")
                nc.vector.tensor_mul(Linv_sb[:], rnum[:], den[:])   # 1/L = den/num
                rden = TP.tile([1, 1], F32, tag="rden")
                nc.vector.reciprocal(rden[:], den[:])
                L_sb = TP.tile([1, 1], F32, tag="lsb")
                nc.vector.tensor_mul(L_sb[:], num[:], rden[:])
                nc.sync.dma_start(L_out[:], L_sb[:])

                # broadcast Linv to a [128,1] column; derive tau columns
                lps = PS.tile([128, 1], F32, tag="bc1")
                nc.tensor.matmul(lps[:], ones_row[:], Linv_sb[:], start=True, stop=True)
                Linv_col = CP.tile([128, 1], F32, tag="linvcol")
                nc.scalar.activation(Linv_col[:], lps[:], AF.Copy)
                negtau = CP.tile([128, 1], F32, tag="negtau")
                nc.vector.tensor_scalar(negtau[:], Linv_col[:], -LAM, None, OP.mult)
                negLinv = CP.tile([128, 1], F32, tag="neglinv")
                nc.vector.tensor_scalar(negLinv[:], Linv_col[:], -1.0, None, OP.mult)

                # ---------------- A = I - DtD/L ----------------
                A_lo = CP.tile([128, PAD], F32, tag="alo")
                A_hi = CP.tile([64, PAD], F32, tag="ahi")
                nc.vector.scalar_tensor_tensor(A_lo[:], DtD_lo[:], negLinv[:], ID_lo[:], OP.mult, OP.add)
                nc.vector.scalar_tensor_tensor(A_hi[:], DtD_hi[:], negLinv[0:64, :], ID_hi[:], OP.mult, OP.add)

                # ---------------- b = DtY/L (per chunk) ----------------
                b_t = {}
                for c in range(2):
                    sl = slice(c * FC, (c + 1) * FC)
                    pb0 = PS.tile([128, FC], F32, tag="sq0")
                    nc.tensor.matmul(pb0[:], D_sb[:, 0:128], Y_sb[:, sl], start=True, stop=True)
                    b_lo = CP.tile([128, FC], F32, tag=f"blo{c}")
                    nc.vector.tensor_scalar(b_lo[:], pb0[:], Linv_col[:], None, OP.mult)
                    pb1 = PS.tile([64, FC], F32, tag="sq1")
                    nc.tensor.matmul(pb1[:], D_sb[:, 128:PAD], Y_sb[:, sl], start=True, stop=True)
                    b_hi = CP.tile([64, FC], F32, tag=f"bhi{c}")
                    nc.vector.tensor_scalar(b_hi[:], pb1[:], Linv_col[0:64, :], None, OP.mult)
                    b_t[("lo", c)] = b_lo
                    b_t[("hi", c)] = b_hi

            # ---------------- FISTA rounds ----------------
            BLKS = (("lo", 128), ("hi", 64))

            with tc.tile_pool(name="psl", bufs=2, space="PSUM") as PL, \
                 tc.For_i(0, N_REPEAT, 1):
                wl_t = {}     # per-element thresholds for round 2
                x_t = {}
                w_t = {}

                for rnd in range(2):
                    # --- iteration 0: u = b ---
                    for c in range(2):
                        for blk, bp in BLKS:
                            bb = b_t[(blk, c)]
                            xn = SP.tile([bp, FC], F32, tag=f"x{blk}{c}")
                            wn = SP.tile([bp, FC], F32, tag=f"w{blk}{c}")
                            if rnd == 0:
                                r1 = TP.tile([bp, FC], F32, tag=f"p{blk}{c}")
                                nc.scalar.activation(r1[:], bb[:], AF.Relu, bias=negtau[0:bp, :], scale=1.0)
                                r2 = TP.tile([bp, FC], F32, tag=f"q{blk}{c}")
                                nc.scalar.activation(r2[:], bb[:], AF.Relu, bias=negtau[0:bp, :], scale=-1.0)
                                nc.gpsimd.tensor_sub(xn[:], r1[:], r2[:])
                            else:
                                sg = TP.tile([bp, FC], F32, tag=f"p{blk}{c}")
                                nc.scalar.activation(sg[:], bb[:], AF.Sign)
                                av = TP.tile([bp, FC], F32, tag=f"q{blk}{c}")
                                nc.scalar.activation(av[:], bb[:], AF.Abs)
                                rr = TP.tile([bp, FC], F32, tag=f"r{blk}{c}")
                                nc.gpsimd.tensor_sub(rr[:], av[:], wl_t[(blk, c)][:])
                                nc.vector.scalar_tensor_tensor(xn[:], rr[:], 0.0, sg[:], OP.max, OP.mult)
                            nc.vector.tensor_scalar(wn[:], xn[:], -1.0, None, OP.mult)
                            x_t[(blk, c)] = xn
                            w_t[(blk, c)] = wn

                    # --- iterations 1..N-1 ---
                    for k in range(1, N_ITERS):
                        al = alphas[k - 1]  # -(1+tt_{k-1})
                        sk = ss[k]          # tt_k/(1+tt_k)
                        for c in range(2):
                            P0 = PL.tile([128, FC], F32, tag=f"P0{c}")
                            nc.tensor.matmul(P0[:], A_lo[:, 0:128], w_t[("lo", c)][:], start=True, stop=False)
                            nc.tensor.matmul(P0[:], A_hi[:, 0:128], w_t[("hi", c)][:], start=False, stop=True)
                            P1 = PL.tile([64, FC], F32, tag=f"P1{c}")
                            nc.tensor.matmul(P1[:], A_lo[:, 128:PAD], w_t[("lo", c)][:], start=True, stop=False)
                            nc.tensor.matmul(P1[:], A_hi[:, 128:PAD], w_t[("hi", c)][:], start=False, stop=True)
                            # Both u passes are emitted before the shrink chains so
                            # the hi block's chain is not queued on DVE behind the
                            # lo block's tail (DVE executes its queue in order).
                            u_b = {}
                            for (blk, bp), P in zip(BLKS, (P0, P1)):
                                u = TP.tile([bp, FC], F32, tag=f"u{blk}{c}")
                                nc.vector.scalar_tensor_tensor(u[:], P[:], al, b_t[(blk, c)][:], OP.mult, OP.add)
                                u_b[blk] = u
                            for blk, bp in BLKS:
                                u = u_b[blk]
                                xn = SP.tile([bp, FC], F32, tag=f"x{blk}{c}")
                                if rnd == 0:
                                    r1 = TP.tile([bp, FC], F32, tag=f"p{blk}{c}")
                                    nc.scalar.activation(r1[:], u[:], AF.Relu, bias=negtau[0:bp, :], scale=1.0)
                                    r2 = TP.tile([bp, FC], F32, tag=f"q{blk}{c}")
                                    nc.scalar.activation(r2[:], u[:], AF.Relu, bias=negtau[0:bp, :], scale=-1.0)
                                    nc.gpsimd.tensor_sub(xn[:], r1[:], r2[:])
                                else:
                                    sg = TP.tile([bp, FC], F32, tag=f"p{blk}{c}")
                                    nc.scalar.activation(sg[:], u[:], AF.Sign)
                                    av = TP.tile([bp, FC], F32, tag=f"q{blk}{c}")
                                    nc.scalar.activation(av[:], u[:], AF.Abs)
                                    rr = TP.tile([bp, FC], F32, tag=f"r{blk}{c}")
                                    nc.gpsimd.tensor_sub(rr[:], av[:], wl_t[(blk, c)][:])
                                    if c == 0:
                                        nc.vector.scalar_tensor_tensor(xn[:], rr[:], 0.0, sg[:], OP.max, OP.mult)
                                    else:
                                        rr2 = TP.tile([bp, FC], F32, tag=f"s{blk}{c}")
                                        nc.scalar.activation(rr2[:], rr[:], AF.Relu)
                                        nc.gpsimd.tensor_mul(xn[:], rr2[:], sg[:])
                                wn = SP.tile([bp, FC], F32, tag=f"w{blk}{c}")
                                nc.vector.scalar_tensor_tensor(wn[:], x_t[(blk, c)][:], sk, xn[:], OP.mult, OP.subtract)
                                x_t[(blk, c)] = xn
                                w_t[(blk, c)] = wn

                    # --- between rounds: reweighting -> wl ---
                    if rnd == 0:
                        for c in range(2):
                            sl = slice(c * FC, (c + 1) * FC)
                            nc.sync.dma_start(C1_out[0:128, sl], x_t[("lo", c)][:])
                            nc.sync.dma_start(C1_out[128:NP_, sl], x_t[("hi", c)][0:33, :])
                        for c in range(2):
                            sl2 = slice(c * FC, (c + 1) * FC)
                            wr = {}
                            for blk, bp in BLKS:
                                t1 = TP.tile([bp, FC], F32, tag=f"p{blk}{c}")
                                nc.vector.scalar_tensor_tensor(t1[:], x_t[(blk, c)][:], -1.0, x_t[(blk, c)][:], OP.mult, OP.max)
                                nc.vector.tensor_scalar(t1[:], t1[:], 0.01, None, OP.add)
                                w_raw = TP.tile([bp, FC], F32, tag=f"q{blk}{c}")
                                nc.vector.reciprocal(w_raw[:], t1[:])
                                wr[blk] = w_raw
                            sps = PL.tile([1, FC], F32, tag=f"P0{c}")
                            nc.tensor.matmul(sps[:], ones_lo[:], wr["lo"][:], start=True, stop=False)
                            nc.tensor.matmul(sps[:], ones_hi[:], wr["hi"][:], start=False, stop=True)
                            sums = TP.tile([1, FC], F32, tag=f"sums{c}")
                            nc.scalar.activation(sums[:], sps[:], AF.Copy)
                            rsums = TP.tile([1, FC], F32, tag=f"rsums{c}")
                            nc.vector.reciprocal(rsums[:], sums[:])
                            brow = TP.tile([1, FC], F32, tag=f"brow{c}")
                            nc.vector.tensor_scalar(brow[:], rsums[:], Linv_sb[:], float(NP_ * LAM), OP.mult, OP.mult)
                            bc0 = PL.tile([128, FC], F32, tag=f"P1{c}")
                            nc.tensor.matmul(bc0[:], ones_row[:], brow[:], start=True, stop=True)
                            wl_lo = CP.tile([128, FC], F32, tag=f"wllo{c}")
                            nc.vector.tensor_mul(wl_lo[:], wr["lo"][:], bc0[:])
                            bc1 = PL.tile([64, FC], F32, tag=f"P0{c}")
                            nc.tensor.matmul(bc1[:], ones_row[:, 0:64], brow[:], start=True, stop=True)
                            wl_hi = CP.tile([64, FC], F32, tag=f"wlhi{c}")
                            nc.vector.tensor_mul(wl_hi[:], wr["hi"][:], bc1[:])
                            wl_t[("lo", c)] = wl_lo
                            wl_t[("hi", c)] = wl_hi
                            nc.sync.dma_start(WL_out[0:128, sl2], wl_lo[:])
                            nc.sync.dma_start(WL_out[128:NP_, sl2], wl_hi[0:33, :])

                # ---------------- outputs ----------------
                for c in range(2):
                    sl = slice(c * FC, (c + 1) * FC)
                    nc.sync.dma_start(C_out[0:128, sl], x_t[("lo", c)][:])
                    nc.sync.dma_start(C_out[128:NP_, sl], x_t[("hi", c)][0:33, :])
                    rps = PL.tile([T, FC], F32, tag=f"P1{c}")
                    nc.tensor.matmul(rps[:], Dt_lo[:], x_t[("lo", c)][:], start=True, stop=False)
                    nc.tensor.matmul(rps[:], Dt_hi[:], x_t[("hi", c)][:], start=False, stop=True)
                    rsb = TP.tile([T, FC], F32, tag=f"rsb{c}")
                    nc.scalar.activation(rsb[:], rps[:], AF.Copy)
                    nc.sync.dma_start(R_out[:, sl], rsb[:])

    nc.compile()
    return nc


def kernel(x, rho, theta):
    x = np.ascontiguousarray(x, dtype=np.float32)
    rho = np.ascontiguousarray(rho, dtype=np.float32)
    theta = np.ascontiguousarray(theta, dtype=np.float32)

    nc = _build_module()

    rng = np.random.default_rng(42)
    r0 = rng.standard_normal((PAD, 1)).astype(np.float32)
    id192 = np.eye(PAD, dtype=np.float32)
    in_maps = []
    for i in range(N_CORES):
        shard = x[i * NSH:(i + 1) * NSH]                      # [16, 36, 50]
        Y = np.ascontiguousarray(shard.transpose(1, 0, 2).reshape(T, F))
        in_maps.append({
            "Y_in": Y,
            "rho_in": rho.reshape(80, 1),
            "th_in": theta.reshape(80, 1),
            "id_in": id192,
            "r0_in": r0,
        })

    trace = bool(int(os.environ.get("KBENCH_TRACE", "0")))
    res = run_bass_kernel_spmd(nc, in_maps, list(range(N_CORES)), trace=trace)
    global LAST_EXEC_NS, LAST_RESULTS
    LAST_EXEC_NS = res.exec_time_ns
    LAST_RESULTS = res

    C = np.empty((NS, NP_, DD), np.float32)
    R = np.empty((NS, T, DD), np.float32)
    for i in range(N_CORES):
        r = res.results[i]
        C[i * NSH:(i + 1) * NSH] = r["C_out"].reshape(NP_, NSH, DD).transpose(1, 0, 2)
        R[i * NSH:(i + 1) * NSH] = r["R_out"].reshape(T, NSH, DD).transpose(1, 0, 2)
    D = res.results[0]["D_out"]
    return C, D, R
